# revision 92
# baseline (speedup 1.0000x reference)
"""DSQG sparse attention kernel for 8 Trainium2 NeuronCores.

Problem: B=2, T=2048, C=768, H=12, HD=64, J=52 offsets (41 dense 0..40 + 11 sparse).
out = softmax_j(q . (k[t-oj] * (1+se[j])) / 8 + pb[j,h]) @ v[t-oj], then out-proj.

Sharding (SPMD, one program, 8 input sets):
  core c: b = c//4, th = (c%4)//2 (T-half), hg = (c%4)%2 (head-group of 6).
  Queries t in [th*1024, th*1024+1024), K/V halo [t0-384, t0+1024) zero-padded.
  Host sums the hg partials per (b, th) and concatenates.

Per-core pipeline (engine-balanced, software-pipelined; ~142us TimelineSim):
  P1 PE  : qk-proj -> per-pack QT [128=(2h x 64d), 1024] / KT [128, 1408] bf16
           (kc-outer over 5 concurrent PSUM tiles, paced by kc-chunked input
           DMAs); v-proj -> V [t%128, blk, d']. All but pack0's QK and half
           the v-proj are interleaved into later score loops.
  P2 DVE : per offset j: prod_j = QT .* KT[:, shifted]  (bf16 TT, 2x mode;
           a few js run on the idle Pool engine instead)
     PE  : scores += dse_j.T @ prod_j (32-col lhsT, zero-padded, PSUM [128,512])
     DVE : validity mask = min(scores, vmask) on the h0 PSUM half (pre-exp)
     ACT : EP = exp(scores/8 + pb) bf16
  Chain per (pack, tau):
     PE  : transpose EP tile -> PT [t, 104+pad] (+ ACT/DVE PSUM->SBUF copy)
     DVE : row-sums per head (tensor_reduce) -> reciprocal
     ACT : PTn = PT * rec[t,h] (per-partition scale) = normalized probs
     GPS : local_scatter PTn -> double band [t, 1024] (both heads, diagonal)
     DMA : xbar-transpose band -> bt [w, 8, 128] chunks (SP queue; the last
           four tail chains use PE transposes + DVE copies instead)
  P6 PE  : AV: psAV[128,:] += V-chunk.T @ bt-chunk (h0 rows 0:64, h1 64:128)
     ACT : copy psAV -> OHT[p] bf16
  P7 PE  : out-proj OUT[t, 768] = OHT.T @ WoT (if_gain folded); f32 out DMA
           per 512/256-col half on the SP queue.
  Chain/AV/out-proj work of pack p-1 is interleaved at tuned offsets into
  pack p's j-loop so PE/DVE rarely stall; pack 2 is split into two 512-wide
  halves so the last softmax chains start earlier and the tail stays short.
"""
import sys
sys.path.insert(0, "/opt/trn_rl_repo")

import numpy as np
import ml_dtypes

BF16 = ml_dtypes.bfloat16

B, T, C, H, HD = 2, 2048, 768, 12, 64
J = 52
OFFS = np.array(list(range(41)) + [96, 128, 145, 163, 185, 209, 236, 266, 301, 340, 384],
                dtype=np.int32)
NUM_LOCAL_HEADS = 7
DISTAL_THRESHOLD = 350.0
TQ = 1024          # queries per core
HALO = 384
TK = TQ + HALO     # 1408
HPC = 6            # heads per core
NPACK = 3          # head pairs per core

_compiled = None


def _build():
    import concourse.bass as bass
    import concourse.tile as tile
    from concourse import mybir, bacc
    from concourse.masks import make_identity

    nc = bacc.Bacc()
    f32, bf16, i16 = mybir.dt.float32, mybir.dt.bfloat16, mybir.dt.int16
    XY = mybir.AxisListType.XY
    ADD = mybir.AluOpType.add

    xt = nc.dram_tensor("xt", [768, TK], bf16, kind="ExternalInput")
    wqk = nc.dram_tensor("wqk", [768, 768], bf16, kind="ExternalInput")
    wv = nc.dram_tensor("wv", [768, 384], bf16, kind="ExternalInput")
    wo = nc.dram_tensor("wo", [384, 768], bf16, kind="ExternalInput")
    dse = nc.dram_tensor("dse", [128, J * 32], bf16, kind="ExternalInput")
    pb = nc.dram_tensor("pb", [128, NPACK], f32, kind="ExternalInput")
    vmask = nc.dram_tensor("vmask", [128, 512], f32, kind="ExternalInput")
    sidx = nc.dram_tensor("sidx", [128, 128], i16, kind="ExternalInput")
    out_d = nc.dram_tensor("out", [TQ, 768], f32, kind="ExternalOutput")

    NT = TQ // 128   # 8 query tiles
    NB = TK // 128   # 11 halo blocks
    Q_RANGES = [(384, 896), (896, 1408)]
    K_RANGES = [(0, 512), (512, 1024), (1024, 1408)]

    with tile.TileContext(nc) as tc:
        import contextlib
        with contextlib.ExitStack() as ctx:
            consts = ctx.enter_context(tc.tile_pool(name="consts", bufs=1))
            qkv = ctx.enter_context(tc.tile_pool(name="qkv", bufs=1))
            prodp = ctx.enter_context(tc.tile_pool(name="prod", bufs=24))
            prodhp = ctx.enter_context(tc.tile_pool(name="prodh", bufs=24))
            epp = ctx.enter_context(tc.tile_pool(name="ep", bufs=1))
            ptp = ctx.enter_context(tc.tile_pool(name="pt", bufs=10))
            ptnp = ctx.enter_context(tc.tile_pool(name="ptn", bufs=10))
            bandp = ctx.enter_context(tc.tile_pool(name="band", bufs=10))
            btp = ctx.enter_context(tc.tile_pool(name="bandT", bufs=10))
            ohp = ctx.enter_context(tc.tile_pool(name="oh", bufs=1))
            outp = ctx.enter_context(tc.tile_pool(name="outsb", bufs=3))
            smallp = ctx.enter_context(tc.tile_pool(name="small", bufs=10))
            psS = ctx.enter_context(tc.tile_pool(name="psS", bufs=3, space="PSUM"))
            psAV = ctx.enter_context(tc.tile_pool(name="psAV", bufs=1, space="PSUM"))
            psA = ctx.enter_context(tc.tile_pool(name="psA", bufs=3, space="PSUM"))
            psT = ctx.enter_context(tc.tile_pool(name="psT", bufs=1, space="PSUM"))

            # ---- load constants, kc-chunked so P1 starts while streaming ----
            xt_r = xt.rearrange("(a p) t -> p a t", p=128)
            wqk_r = wqk.rearrange("(a p) m -> p a m", p=128)
            wqk_sb = consts.tile([128, 6, 768], bf16)
            # pack0's Q (mt=0) and K (mt=3) weight columns first
            nc.sync.dma_start(out=wqk_sb[:, :, 0:128], in_=wqk_r[:, :, 0:128])
            nc.sync.dma_start(out=wqk_sb[:, :, 384:512], in_=wqk_r[:, :, 384:512])
            xt_a = consts.tile([128, 3, TK], bf16)
            xt_b = consts.tile([128, 3, TK], bf16)
            for kc in range(6):
                if kc < 3:
                    nc.sync.dma_start(out=xt_a[:, kc], in_=xt_r[:, kc])
                else:
                    nc.sync.dma_start(out=xt_b[:, kc - 3], in_=xt_r[:, kc])
            nc.sync.dma_start(out=wqk_sb[:, :, 128:384], in_=wqk_r[:, :, 128:384])
            nc.sync.dma_start(out=wqk_sb[:, :, 512:768], in_=wqk_r[:, :, 512:768])
            dse_sb = consts.tile([128, J, 32], bf16)
            nc.sync.dma_start(out=dse_sb, in_=dse.rearrange("p (j m) -> p j m", m=32))
            pb_sb = consts.tile([128, NPACK], f32)
            nc.sync.dma_start(out=pb_sb, in_=pb[:])
            vmask_sb = consts.tile([128, 512], f32)
            nc.sync.dma_start(out=vmask_sb, in_=vmask[:])
            sidx_sb = consts.tile([128, 128], i16)
            nc.sync.dma_start(out=sidx_sb, in_=sidx[:])
            wv_sb = consts.tile([128, 6, 384], bf16)
            nc.sync.dma_start(out=wv_sb, in_=wv.rearrange("(a p) m -> p a m", p=128))
            wo_sb = consts.tile([128, 3, 768], bf16)
            nc.sync.dma_start(out=wo_sb, in_=wo.rearrange("(a p) m -> p a m", p=128))
            ident = consts.tile([128, 128], bf16)
            make_identity(nc, ident)

            def xt_sb(kc):
                return xt_a[:, kc] if kc < 3 else xt_b[:, kc - 3]

            QT = [qkv.tile([128, TQ], bf16, name=f"QT{i}") for i in range(NPACK)]
            KT = [qkv.tile([128, TK], bf16, name=f"KT{i}") for i in range(NPACK)]
            V = qkv.tile([128, NB, 384], bf16)
            EP = [epp.tile([128, TQ], bf16, name=f"EP{i}") for i in range(NPACK)]
            OHT = [ohp.tile([128, TQ], bf16, name=f"OHT{i}") for i in range(NPACK)]

            # ---- emission helpers ----
            bt_tiles = {}

            def qk_chunk(p, mt, n0, n1, dve_copy=False):
                nw = n1 - n0
                ps = psA.tile([128, 512], f32, tag="psA")
                for kc in range(6):
                    nc.tensor.matmul(
                        ps[:, 0:nw],
                        wqk_sb[:, kc, mt * 128:(mt + 1) * 128],
                        xt_sb(kc)[:, n0:n1],
                        start=(kc == 0), stop=(kc == 5))
                dst = (QT[p][:, n0 - 384:n1 - 384] if mt < 3
                       else KT[p][:, n0:n1])
                if dve_copy:
                    nc.vector.tensor_copy(dst, ps[:, 0:nw])
                else:
                    nc.scalar.copy(dst, ps[:, 0:nw])

            def v_chunk(tt):
                ps = psA.tile([128, 512], f32, tag="psA")
                for kc in range(6):
                    nc.tensor.matmul(
                        ps[:, 0:384],
                        xt_sb(kc)[:, tt * 128:(tt + 1) * 128],
                        wv_sb[:, kc, :],
                        start=(kc == 0), stop=(kc == 5))
                nc.scalar.copy(V[:, tt, :], ps[:, 0:384])

            def emit_mask(sps):
                # validity mask as a pre-exp min on the PSUM scores: invalid
                # (t < offset) entries clamp to -1e5 so exp flushes to zero.
                # Runs right after the last score matmul, so downstream
                # transposes depend only on exp.
                nc.vector.tensor_tensor(
                    sps[:], sps[:], vmask_sb[:], mybir.AluOpType.min)

            def emit_exp(p, sps, n):
                nc.scalar.activation(
                    EP[p][:, n * 512:(n + 1) * 512], sps[:],
                    mybir.ActivationFunctionType.Exp,
                    bias=pb_sb[:, p:p + 1], scale=0.125)

            def emit_ept_pe(p, taus, act_copy=False):
                # EP transposes on PE + DVE/ACT copies (no DMA round trip);
                # one PSUM alloc covers up to 8 taus
                pst = psT.tile([128, 1024], bf16, tag="psT")
                for i, tau in enumerate(taus):
                    nc.tensor.transpose(
                        pst[:, i * 128:(i + 1) * 128],
                        EP[p][:, tau * 128:(tau + 1) * 128], ident)
                for i, tau in enumerate(taus):
                    pt = ptp.tile([128, 4, 32], bf16, tag="pt")
                    flat = pt[:].rearrange("p a b -> p (a b)")
                    if act_copy:
                        nc.scalar.copy(flat, pst[:, i * 128:(i + 1) * 128])
                    else:
                        nc.vector.tensor_copy(flat, pst[:, i * 128:(i + 1) * 128])
                    bt_tiles[(p, tau, "pt")] = pt

            def emit_tail(p, tau, pe_bandT=False):
                # reduce -> recip -> normalize -> scatter -> band transpose
                pt = bt_tiles.pop((p, tau, "pt"))
                sums = smallp.tile([128, 2], f32, tag="sums")
                for h in range(2):
                    nc.vector.tensor_reduce(
                        sums[:, h:h + 1], pt[:, :, 13 * h:13 * h + 13],
                        axis=XY, op=ADD)
                rec = smallp.tile([128, 2], f32, tag="rec")
                nc.vector.reciprocal(rec, sums)
                ptn = ptnp.tile([128, 4, 32], bf16, tag="ptn")
                nc.scalar.activation(
                    ptn[:, :, 0:13], pt[:, :, 0:13],
                    mybir.ActivationFunctionType.Copy, scale=rec[:, 0:1])
                nc.scalar.activation(
                    ptn[:, :, 13:32], pt[:, :, 13:32],
                    mybir.ActivationFunctionType.Copy, scale=rec[:, 1:2])
                band = bandp.tile([128, 1024], bf16, tag="band")
                nc.gpsimd.local_scatter(
                    out_ap=band[:], data_ap=ptn[:], idxs_ap=sidx_sb[:],
                    channels=128, num_elems=1024, num_idxs=128)
                bt = btp.tile([128, 8, 128], bf16, tag="bt")
                if pe_bandT:
                    # tail fast-path: PE transposes (low latency, PE is idle
                    # here) + DVE copy instead of the 3us DMA round trip
                    pst = psT.tile([128, 1024], bf16, tag="psT")
                    for c in range(8):
                        nc.tensor.transpose(
                            pst[:, c * 128:(c + 1) * 128],
                            band[:, c * 128:(c + 1) * 128], ident)
                    nc.vector.tensor_copy(
                        bt[:].rearrange("p a b -> p (a b)"), pst[:])
                else:
                    nc.sync.dma_start(out=bt[:], in_=band[:], transpose=True)
                bt_tiles[(p, tau)] = bt

            def emit_av(p, tau, dve_copy=False):
                bt = bt_tiles.pop((p, tau))
                ps = psAV.tile([128, 128], f32, tag="psAV")
                for h in range(2):
                    for cch in range(4):
                        nc.tensor.matmul(
                            ps[64 * h:64 * h + 64, :],
                            V[:, tau + cch, 64 * (2 * p + h):64 * (2 * p + h) + 64],
                            bt[:, 4 * h + cch, :],
                            start=(cch == 0), stop=(cch == 3))
                dst = OHT[p][:, tau * 128:(tau + 1) * 128]
                if dve_copy:
                    nc.vector.tensor_copy(dst, ps[:])
                else:
                    nc.scalar.copy(dst, ps)

            def emit_p7(tau, dve_osb=False):
                # dve_osb: in the tail the ACT queue carries the critical
                # norm chain; osb copies go to the idle DVE instead
                osb = outp.tile([128, 768], f32, tag="osb")
                for (n0, n1) in [(0, 512), (512, 768)]:
                    nw = n1 - n0
                    ps = psA.tile([128, 512], f32, tag="psA")
                    for g in range(3):
                        nc.tensor.matmul(
                            ps[:, 0:nw],
                            OHT[g][:, tau * 128:(tau + 1) * 128],
                            wo_sb[:, g, n0:n1],
                            start=(g == 0), stop=(g == 2))
                    if dve_osb:
                        nc.vector.tensor_copy(osb[:, n0:n1], ps[:, 0:nw])
                    else:
                        nc.scalar.copy(osb[:, n0:n1], ps[:, 0:nw])
                    nc.sync.dma_start(
                        out=out_d[tau * 128:(tau + 1) * 128, n0:n1],
                        in_=osb[:, n0:n1])

            def p2_loop_full(p, sched, pool_js=()):
                """52 offsets, full 1024-wide prods, two matmul halves each.
                Js in pool_js compute their product on the (idle) Pool engine
                to take load off DVE, the window's bottleneck."""
                sps0 = psS.tile([128, 512], f32, tag="psS")
                sps1 = psS.tile([128, 512], f32, tag="psS")
                for jj in range(J):
                    for fn in sched.get(jj, ()):
                        fn()
                    cg, q = jj // 13, jj % 13
                    oj = int(OFFS[jj])
                    prod = prodp.tile([128, TQ], bf16, tag="prod")
                    eng = nc.gpsimd if jj in pool_js else nc.vector
                    eng.tensor_mul(
                        prod, QT[p][:], KT[p][:, HALO - oj:HALO - oj + TQ])
                    for n in range(2):
                        nc.tensor.matmul(
                            sps1[32 * cg:32 * cg + 32, :] if n else
                            sps0[32 * cg:32 * cg + 32, :],
                            dse_sb[:, jj, :],
                            prod[:, n * 512:(n + 1) * 512],
                            start=(q == 0), stop=(q == 12),
                            tile_position=(0, 32 * cg))
                return sps0, sps1

            def p2_loop_half(p, n, sched, pool_js=()):
                """52 offsets, one 512-wide half."""
                sps = psS.tile([128, 512], f32, tag="psS")
                for jj in range(J):
                    for fn in sched.get(jj, ()):
                        fn()
                    cg, q = jj // 13, jj % 13
                    oj = int(OFFS[jj])
                    prod = prodhp.tile([128, 512], bf16, tag="prodh")
                    eng = nc.gpsimd if jj in pool_js else nc.vector
                    eng.tensor_mul(
                        prod,
                        QT[p][:, n * 512:(n + 1) * 512],
                        KT[p][:, HALO - oj + n * 512:HALO - oj + n * 512 + 512])
                    nc.tensor.matmul(
                        sps[32 * cg:32 * cg + 32, :],
                        dse_sb[:, jj, :],
                        prod[:],
                        start=(q == 0), stop=(q == 12),
                        tile_position=(0, 32 * cg))
                return sps

            def add(sched, jj, fn):
                sched.setdefault(jj, []).append(fn)

            # ---- pack0 QK projection, then pipelined everything ----
            # kc-outer with five concurrent PSUM tiles: PE consumes each
            # arriving xt chunk for all five output tiles, so every tile
            # stops right after the last xt chunk lands instead of
            # cascading. K accumulates in the (still unused) psS ring.
            p0chunks = [(0, r, psA) for r in Q_RANGES] + \
                       [(3, r, psS) for r in K_RANGES]
            p0ps = [pool.tile([128, 512], f32, name=f"p0ps{i}",
                              tag=("psA" if pool is psA else "psS"))
                    for i, (_, _, pool) in enumerate(p0chunks)]
            for kc in range(6):
                for (mt, (n0, n1), _), ps in zip(p0chunks, p0ps):
                    nc.tensor.matmul(
                        ps[:, 0:n1 - n0],
                        wqk_sb[:, kc, mt * 128:(mt + 1) * 128],
                        xt_sb(kc)[:, n0:n1],
                        start=(kc == 0), stop=(kc == 5))
            for i, ((mt, (n0, n1), _), ps) in enumerate(zip(p0chunks, p0ps)):
                dst = (QT[0][:, n0 - 384:n1 - 384] if mt < 3
                       else KT[0][:, n0:n1])
                if i % 2 == 0:
                    nc.vector.tensor_copy(dst, ps[:, 0:n1 - n0])
                else:
                    nc.scalar.copy(dst, ps[:, 0:n1 - n0])

            # pack 0 scores; remaining projections stream through the gaps
            sched = {}
            chunks = ([(1, 1, r) for r in Q_RANGES] + [(1, 4, r) for r in K_RANGES]
                      + [(2, 2, r) for r in Q_RANGES] + [(2, 5, r) for r in K_RANGES])
            for i, (p_, mt_, r_) in enumerate(chunks):
                add(sched, 4 * i, lambda p_=p_, mt_=mt_, r_=r_: qk_chunk(p_, mt_, *r_))
            for i in range(6):
                add(sched, 40 + 2 * i, lambda tt=i: v_chunk(tt))
            sps0, sps1 = p2_loop_full(0, sched, pool_js={38, 40, 42, 44, 46, 48, 50})
            emit_mask(sps0)
            emit_exp(0, sps0, 0)
            emit_exp(0, sps1, 1)

            # pack 1 scores; pack0 softmax chains + AV interleaved.
            # The last AV bundle comes after the exps so they reach the ACT
            # queue (and free the psS ring) without waiting on a late AV copy.
            sched = {}
            add(sched, 3, lambda: emit_ept_pe(0, range(NT), act_copy=True))
            # v-proj blocks 6..10 land here: the pack0 window is PE-bound,
            # this one is DVE-bound with PE slack
            for i in range(5):
                add(sched, 1 + 4 * i, lambda tt=6 + i: v_chunk(tt))
            for t in range(NT - 1):
                add(sched, 7 + 3 * t, lambda t=t: emit_tail(0, t))
                add(sched, 30 + 3 * t, lambda t=t: emit_av(0, t))
            add(sched, 28, lambda: emit_tail(0, 7))
            sps0, sps1 = p2_loop_full(1, sched, pool_js={1, 3, 5, 7, 9, 11})
            emit_mask(sps0)
            emit_exp(1, sps0, 0)
            emit_exp(1, sps1, 1)
            emit_av(0, 7)

            # pack 2 half 0; ALL pack1 chains/AV interleaved (their inputs
            # are ready; keeps the next loop's Pool budget for pack2)
            sched = {}
            add(sched, 3, lambda: emit_ept_pe(1, range(NT), act_copy=True))
            for t in range(NT):
                add(sched, 8 + 3 * t, lambda t=t: emit_tail(1, t))
                if t < NT - 1:
                    add(sched, 34 + 2 * t,
                        lambda t=t: emit_av(1, t, dve_copy=(t >= 5)))
            sps20 = p2_loop_half(2, 0, sched, pool_js={1, 3, 5, 7})
            emit_mask(sps20)
            emit_exp(2, sps20, 0)
            emit_av(1, 7)

            # pack 2 half 1; pack2 tau 0..3 chains + AV + out-proj 0..2
            sched = {}
            add(sched, 6, lambda: emit_ept_pe(2, range(4), act_copy=True))
            for t in range(4):
                add(sched, 11 + 3 * t, lambda t=t: emit_tail(2, t))
                add(sched, 34 + 3 * t,
                    lambda t=t: emit_av(2, t, dve_copy=(t >= 2)))
            add(sched, 38, lambda: emit_p7(0))
            add(sched, 42, lambda: emit_p7(1))
            sps21 = p2_loop_half(2, 1, sched, pool_js={1, 3, 30, 34, 38, 42, 46, 50})
            # exp first on ACT; EP transposes for the last taus on PE,
            # so the tau 4..7 chains launch as soon as exp lands
            emit_exp(2, sps21, 1)
            emit_ept_pe(2, range(4, NT))

            # tail: drain pack2 tau 4..7 chains between out-proj tiles;
            # band transposes on PE (it is idle here and latency matters)
            emit_p7(2)
            emit_tail(2, 4, pe_bandT=True)
            emit_p7(3)
            emit_tail(2, 5, pe_bandT=True)
            emit_av(2, 4, dve_copy=True)
            emit_p7(4)
            emit_tail(2, 6, pe_bandT=True)
            emit_av(2, 5, dve_copy=True)
            emit_p7(5)
            emit_tail(2, 7, pe_bandT=True)
            emit_av(2, 6, dve_copy=True)
            emit_p7(6)
            emit_av(2, 7, dve_copy=True)
            emit_p7(7)

    nc.compile()
    return nc


def _host_prep(x, W_qkv, W_out, pos_bias, scale_embed, if_gain):
    """Build the 8 per-core input dicts."""
    delta = OFFS.astype(np.float32)
    distal = delta > DISTAL_THRESHOLD
    hidx = np.arange(H)
    pbm = np.where(distal[:, None] & (hidx[None, :] < NUM_LOCAL_HEADS), -10000.0,
                   pos_bias.astype(np.float32))
    pbm = np.where((~distal)[:, None] & (hidx[None, :] >= NUM_LOCAL_HEADS), -3.0, pbm)

    def mrow(jj, h):
        return 32 * (jj // 13) + 13 * h + (jj % 13)

    # sidx[i, m] = 512*h + i + 384 - o_j for m == mrow(j, h) else -1 (ignored)
    sidx_np = np.full((128, 128), -1, dtype=np.int16)
    for h in range(2):
        for jj in range(J):
            sidx_np[:, mrow(jj, h)] = (
                512 * h + np.arange(128) + HALO - OFFS[jj]).astype(np.int16)

    # dse: [128, J*32] lhsT, packed per-j [128, 32] blocks (cols 26-31 zero)
    dse_np = np.zeros((J, 128, 32), dtype=np.float32)
    se1 = 1.0 + scale_embed.astype(np.float32)  # [J, HD]
    for jj in range(J):
        for h in range(2):
            dse_np[jj, h * 64:(h + 1) * 64, 13 * h + (jj % 13)] = se1[jj]
    dse_flat = dse_np.transpose(1, 0, 2).reshape(128, J * 32)

    in_maps = []
    for c in range(8):
        b, q = divmod(c, 4)
        th, hg = divmod(q, 2)
        heads = np.arange(hg * HPC, hg * HPC + HPC)
        t0 = th * TQ

        # xt: [768, TK] halo-padded transpose of x[b]
        xt_np = np.zeros((768, TK), dtype=np.float32)
        lo = t0 - HALO
        src_lo = max(lo, 0)
        xt_np[:, src_lo - lo:] = x[b, src_lo:t0 + TQ, :].T
        # wqk: [768, 768] lhsT; cols 0..383 q-heads, 384..767 k-heads
        qrows = np.concatenate([np.arange(h * HD, (h + 1) * HD) for h in heads])
        wqk_np = np.concatenate(
            [W_qkv[qrows, :].T, W_qkv[768 + qrows, :].T], axis=1)
        wv_np = W_qkv[1536 + qrows, :].T
        # wo: [384, 768] lhsT for out-proj, if_gain folded
        gain = np.repeat(if_gain[heads], HD)
        wo_np = (W_out[:, qrows] * gain[None, :]).T
        # pb: [128, NPACK] bias columns per pack
        pb_np = np.zeros((128, NPACK), dtype=np.float32)
        for p in range(NPACK):
            for h in range(2):
                for jj in range(J):
                    pb_np[mrow(jj, h), p] = pbm[jj, heads[2 * p + h]]
        # vmask [128, 512]: min-clamp for the pre-exp scores; -1e5 where
        # global t < offset (th=0 only), +1e30 (passthrough) elsewhere
        vm = np.full((128, 512), 1e30, dtype=np.float32)
        if th == 0:
            tg = np.arange(512)
            for h in range(2):
                for jj in range(J):
                    vm[mrow(jj, h), :] = np.where(tg >= OFFS[jj], 1e30, -1e5)
        in_maps.append({
            "xt": xt_np.astype(BF16),
            "wqk": wqk_np.astype(BF16),
            "wv": wv_np.astype(BF16),
            "wo": wo_np.astype(BF16),
            "dse": dse_flat.astype(BF16),
            "pb": pb_np,
            "vmask": vm.astype(BF16),
            "sidx": sidx_np,
        })
    return in_maps


def kernel(x, W_qkv, W_out, pos_bias, scale_embed, if_gain):
    global _compiled
    from concourse.bass_utils import run_bass_kernel_spmd

    x = np.asarray(x, dtype=np.float32)
    W_qkv = np.asarray(W_qkv, dtype=np.float32)
    W_out = np.asarray(W_out, dtype=np.float32)
    pos_bias = np.asarray(pos_bias, dtype=np.float32)
    scale_embed = np.asarray(scale_embed, dtype=np.float32)
    if_gain = np.asarray(if_gain, dtype=np.float32)

    if _compiled is None:
        _compiled = _build()
    in_maps = _host_prep(x, W_qkv, W_out, pos_bias, scale_embed, if_gain)
    res = run_bass_kernel_spmd(_compiled, in_maps, core_ids=list(range(8)))

    out = np.zeros((B, T, C), dtype=np.float32)
    for c in range(8):
        b, q = divmod(c, 4)
        th, _ = divmod(q, 2)
        t0 = th * TQ
        out[b, t0:t0 + TQ, :] += res.results[c]["out"]
    return out
